# revision 40
# baseline (speedup 1.0000x reference)
"""CrossViewTransformer Trainium2 kernel.

Math (per batch b):
    q = Wq @ bev + bq          [D=8,  N=9216]
    k = Wk @ rv  + bk          [8,  N]
    v = Wv @ rv  + bv          [64, N]
    E[j, i] = k[:, j] . q[:, i]            (energy, rows=key pixel j, cols=query pixel i)
    A = softmax over i of E[j, :]
    z[:, j] = sum_i A[j, i] * v[:, i]
    out = bev + z

Sharding: 8 cores = 2 batches x 4 j-slabs of 2304 columns. Each core computes
softmax over the full i axis for its j slab; no collectives.

Device layout (per core):
    E^T tiles [i-chunk=128, j-block] from matmul(lhsT=q[:, i-tile], rhs=k[:, jblk])
    P^T = exp(E^T) via ScalarE (logits are O(5), no max subtraction needed), bf16
    Transposed z accumulation: for each j-128-subblock,
        zt[j, c(+denom col)] += matmul(lhsT=P^T[:, jsub], rhs=v^T_ext[i-chunk, 65])
    using P^T as the stationary operand puts j (128 wide) on the output
    partition axis and streams only 65 columns per (i-chunk, jsub) - half the
    PE cycles of the untransposed form, with the softmax denominator riding
    along as vt's ones column (col 64).
    epilogue per jsub: out = zt[:, 0:64] * (1/zt[:, 64]) + bev^T residual,
    DMA'd to a [JS, C] output that the host transposes.

All matmuls in bf16 (1 cycle/col vs 4 for fp32), fp32 PSUM accumulation.
"""

import sys

if "/opt/trn_rl_repo" not in sys.path:
    sys.path.insert(0, "/opt/trn_rl_repo")

import os

import numpy as np
import ml_dtypes

# DVE takes DVE_NUM out of every DVE_DEN exp groups (0 = all on ScalarE)
DVE_NUM = int(os.environ.get("DVE_NUM", "18"))
DVE_DEN = int(os.environ.get("DVE_DEN", "36"))

B, C, H, W = 2, 64, 96, 96
N = H * W            # 9216
D = C // 8           # 8
NT = N // 128        # 72 i-chunks
NCORES = 8
JS = N // 4          # 2304 columns per core
JBLOCKS = [(0, 512), (512, 512), (1024, 512), (1536, 512), (2048, 256)]
GW = 1024            # exp-group width (elements per ACT/DVE call)

BF16 = ml_dtypes.bfloat16

_PROGRAMS = {}


def _build_program(reps=1, dve_share=None):
    dve_num, dve_den = (DVE_NUM, DVE_DEN) if dve_share is None else dve_share
    import concourse.bacc as bacc
    import concourse.mybir as mybir
    from concourse import tile

    F32 = mybir.dt.float32
    BF = mybir.dt.bfloat16
    F8 = mybir.dt.float8e4
    I16 = mybir.dt.int16
    Exp = mybir.ActivationFunctionType.Exp
    Mul = mybir.AluOpType.mult
    Add = mybir.AluOpType.add
    DR = mybir.MatmulPerfMode.DoubleRow
    # bf16 Schraudolph fast-exp constants: bits16 = trunc(x * 128/ln2 + B);
    # int16 bit pattern reinterpreted as bf16 gives exp(x) to ~3% (end-to-end
    # effect ~1e-5 through softmax; validated vs reference). Used to offload a
    # share of the exp work from the bottleneck ScalarE to the DVE.
    EXP_A = float(128.0 / np.log(2.0))
    EXP_B = 16256.0 - 5.0

    nc = bacc.Bacc("TRN2", target_bir_lowering=False, num_devices=NCORES)

    # Single-DMA-friendly layouts: weights concatenated [wq|wk|wv]; the bev
    # residual pre-swizzled on host to SBUF layout [p, (jblk, c)] so it loads
    # (and the output stores) as one contiguous DMA per tensor - each DMA
    # costs ~625ns of serialized HWDGE queue time regardless of size.
    rv_d = nc.dram_tensor("rv_ext", [65, N], BF, kind="ExternalInput")
    bev_d = nc.dram_tensor("bev_ext", [65, N], BF, kind="ExternalInput")
    rvs_d = nc.dram_tensor("rv_slab", [65, JS], BF, kind="ExternalInput")
    bres_d = nc.dram_tensor("bev_res_t", [128, (JS // 128) * C], F32, kind="ExternalInput")
    w_d = nc.dram_tensor("w_ext", [65, 2 * D + 65], BF, kind="ExternalInput")
    out_d = nc.dram_tensor("out", [128, (JS // 128) * C], F32, kind="ExternalOutput")

    with tile.TileContext(nc) as tc:
        with (
            tc.tile_pool(name="const", bufs=1) as cpool,
            tc.tile_pool(name="work", bufs=2) as wpool,
            tc.tile_pool(name="ptile", bufs=6) as ppool,
            tc.tile_pool(name="epi", bufs=4) as xpool,
            tc.tile_pool(name="psum_e", bufs=3, space="PSUM") as epool,
            tc.tile_pool(name="psum_z", bufs=2, space="PSUM") as zpool,
        ):
          for _rep in range(reps):
            # ---- load inputs ----
            rv_sb = cpool.tile([65, N], BF, tag="rv")
            bev_sb = cpool.tile([65, N], BF, tag="bev")
            rvs_sb = cpool.tile([65, JS], BF, tag="rvs")
            # bev residual, transposed: [j within slab, c], grouped by j-128-block
            brest_sb = cpool.tile([128, (JS // 128) * C], F32, tag="brest")
            w_sb = cpool.tile([65, 2 * D + 65], BF, tag="w")
            wq_sb = w_sb[:, 0:D]
            wk_sb = w_sb[:, D : 2 * D]
            wv_sb = w_sb[:, 2 * D : 2 * D + 65]

            # DMA transfers serialize on the shared DMA engine device and
            # each start costs ~625ns of HWDGE queue time, so: chunk the big
            # tensors just enough for early consumption, order by first
            # consumer (q-proj needs bev, k-proj rvs, vt-proj rv), and put
            # brest (needed ~70 groups in) on the parallel SWDGE queue.
            nc.sync.dma_start(w_sb[:], w_d[:])
            nc.sync.dma_start(bev_sb[:, 0:JS], bev_d[:, 0:JS])
            nc.sync.dma_start(rvs_sb[:, 0 : JS // 2], rvs_d[:, 0 : JS // 2])
            nc.sync.dma_start(bev_sb[:, JS : 2 * JS], bev_d[:, JS : 2 * JS])
            nc.sync.dma_start(rvs_sb[:, JS // 2 : JS], rvs_d[:, JS // 2 : JS])
            nc.sync.dma_start(bev_sb[:, 2 * JS : 3 * JS], bev_d[:, 2 * JS : 3 * JS])
            nc.sync.dma_start(bev_sb[:, 3 * JS : N], bev_d[:, 3 * JS : N])
            nc.sync.dma_start(rv_sb[:, 0 : N // 2], rv_d[:, 0 : N // 2])
            nc.sync.dma_start(rv_sb[:, N // 2 : N], rv_d[:, N // 2 : N])
            nc.gpsimd.dma_start(brest_sb[:], bres_d[:])

            # ---- projections ----
            # Copies PSUM->SBUF alternate between DVE and the (idle during
            # prologue) ScalarE to halve the startup critical path.
            # q/k packed for DoubleRow fp8 energy: channel c = s*4 + r lives at
            # partition r, free-slot s (slot-major halves along the free axis).
            q_sb = cpool.tile([4, 2, N], F8, tag="q")    # lhsT tiles for energy
            k_sb = cpool.tile([4, 2, JS], F8, tag="k")   # energy rhs (this core's slab)
            vt_sb = cpool.tile([128, NT * 65], BF, tag="vt")  # v^T_ext chunks

            _ci = [0]

            def pcopy(out, in_):
                if _ci[0] % 5 in (0, 2):
                    nc.vector.tensor_copy(out, in_)
                else:
                    nc.scalar.copy(out, in_)
                _ci[0] += 1

            # Projection producers, invoked lazily: the engine queues are
            # in-order, so emitting all ~63 PSUM->SBUF copies up front would
            # park the first exp behind ~25us of copies. Instead only the
            # first chunks are produced before the main loop; the rest drip
            # in just ahead of their consumer group.
            # Engines may only address PSUM from partition 0, so the two
            # 4-channel halves are projected into two bank-aligned free
            # ranges of one [4, 1024] tile; a single copy then writes the
            # packed [4, 2, 512] fp8 slot-major layout (same element order).
            def k_chunk(cix):
                blk0 = cix * 512
                pw = min(512, JS - blk0)
                ps = epool.tile([4, 1024], F32, tag="e")
                nc.tensor.matmul(
                    ps[:, 0:pw], wk_sb[:, 0:4], rvs_sb[:, blk0 : blk0 + pw],
                    start=True, stop=True,
                )
                nc.tensor.matmul(
                    ps[:, 512 : 512 + pw], wk_sb[:, 4:8],
                    rvs_sb[:, blk0 : blk0 + pw],
                    start=True, stop=True,
                )
                if pw == 512:
                    pcopy(k_sb[:, :, blk0 : blk0 + pw], ps[:])
                else:
                    pcopy(k_sb[:, 0, blk0 : blk0 + pw], ps[:, 0:pw])
                    pcopy(k_sb[:, 1, blk0 : blk0 + pw], ps[:, 512 : 512 + pw])

            def q_chunk(blk):
                s = slice(blk * 512, (blk + 1) * 512)
                ps = epool.tile([4, 1024], F32, tag="e")
                nc.tensor.matmul(
                    ps[:, 0:512], wq_sb[:, 0:4], bev_sb[:, s],
                    start=True, stop=True,
                )
                nc.tensor.matmul(
                    ps[:, 512:1024], wq_sb[:, 4:8], bev_sb[:, s],
                    start=True, stop=True,
                )
                pcopy(q_sb[:, :, s], ps[:])

            def vt_chunk(tg):   # 4 v^T chunks per PSUM tile / copy
                ps = epool.tile([128, 4 * 65], F32, tag="e")
                for m in range(4):
                    t = tg * 4 + m
                    nc.tensor.matmul(
                        ps[:, m * 65 : (m + 1) * 65],
                        rv_sb[:, t * 128 : (t + 1) * 128], wv_sb[:],
                        start=True, stop=True,
                    )
                pcopy(vt_sb[:, tg * 4 * 65 : (tg + 1) * 4 * 65], ps[:])

            kq_done = [0, 0, 0]  # produced counts: k chunks, q chunks, vt tgs

            def produce(nk, nq, nvt):
                while kq_done[0] < min(nk, JS // 512 + 1):
                    k_chunk(kq_done[0]); kq_done[0] += 1
                while kq_done[1] < min(nq, N // 512):
                    q_chunk(kq_done[1]); kq_done[1] += 1
                while kq_done[2] < min(nvt, NT // 4):
                    vt_chunk(kq_done[2]); kq_done[2] += 1

            produce(JS // 512 + 1, N // 512, NT // 4)

            # ---- main attention loop ----
            # Flattened software pipeline across all jblocks: step i issues
            # energy(i), exp(i-1), z(i-3); the next jblock's energy/exp
            # overlaps the previous jblock's z-drain and epilogue. Epilogue
            # work for a jblock is deferred a couple of steps past its last
            # z-matmul (and spread one subblock per step) so reciprocal ops
            # never park in the 4-deep DVE wait queue and stall exp dispatch.
            groups = []
            for jb0, jbw in JBLOCKS:
                g = GW // jbw
                for grp in range(NT // g):
                    groups.append((jb0, jbw, g, grp))
            G = len(groups)

            zt_bufs = {}     # jb0 -> zt psum tile
            o_bufs = {}      # jb0 -> epilogue output tile
            e_tiles = {}
            p_tiles = {}
            epi_sched = {}   # step -> list of (jb0, jbw, s)

            def z_mms(p_tile, i):
                jb0, jbw, g, grp = groups[i]
                zt_buf = zt_bufs[jb0]
                nsub = jbw // 128
                if p_tile.dtype == I16:
                    p_tile = p_tile.bitcast(BF)
                for m in range(g):
                    t = grp * g + m
                    for s in range(nsub):
                        # One accumulation group per PSUM bank (2KB zero
                        # region): start zeroes the whole bank, so only
                        # the bank's first slice starts / last slice stops.
                        nc.tensor.matmul(
                            zt_buf[:, s * 65 : s * 65 + 65],
                            p_tile[:, m * jbw + s * 128 : m * jbw + (s + 1) * 128],
                            vt_sb[:, t * 65 : (t + 1) * 65],
                            start=(t == 0 and s == 0),
                            stop=(t == NT - 1 and s == nsub - 1),
                        )

            nsteps = G + 3 + 3 + max(jbw // 128 for _, jbw in JBLOCKS)
            for i in range(nsteps):
                if i < G:
                    jb0, jbw, g, grp = groups[i]
                    if grp == 0:
                        zt_new = zpool.tile([128, 512], F32, tag="zt")
                        zt_bufs[jb0] = zt_new
                    e_ps = epool.tile([128, GW], F32, tag="e")
                    e_tiles[i] = e_ps
                    for m in range(g):
                        t = grp * g + m
                        for pc0 in range(0, jbw, 256):
                            pw = min(256, jbw - pc0)
                            nc.tensor.matmul(
                                e_ps[:, m * jbw + pc0 : m * jbw + pc0 + pw],
                                q_sb[:, :, t * 128 : (t + 1) * 128],
                                k_sb[:, :, jb0 + pc0 : jb0 + pc0 + pw],
                                start=True, stop=True,
                                perf_mode=DR,
                            )
                if 0 <= i - 1 < G:
                    e_prev = e_tiles.pop(i - 1)
                    if dve_num and ((i - 1) * dve_num) % dve_den < dve_num:
                        # DVE fast-exp: (E*A+B) -> int16 -> bf16 bit pattern
                        p_sb = ppool.tile([128, GW], I16, tag="p")
                        nc.vector.tensor_scalar(
                            p_sb[:], e_prev[:], EXP_A, EXP_B, Mul, Add,
                        )
                    else:
                        p_sb = ppool.tile([128, GW], BF, tag="p")
                        nc.scalar.activation(p_sb[:], e_prev[:], Exp)
                    p_tiles[i - 1] = p_sb
                if 0 <= i - 3 < G:
                    z_mms(p_tiles.pop(i - 3), i - 3)
                    jb0, jbw, g, grp = groups[i - 3]
                    if grp == NT // g - 1:  # jblock's z complete
                        for s in range(jbw // 128):
                            epi_sched.setdefault(i + 2 + s, []).append(
                                (jb0, jbw, s)
                            )
                # ---- normalize + residual + store (per j-128-subblock) ----
                for jb0, jbw, s in epi_sched.pop(i, ()):
                    nsub = jbw // 128
                    zt = zt_bufs[jb0][:, s * 65 : s * 65 + 65]
                    jb = jb0 // 128 + s
                    if s == 0:
                        o_new = xpool.tile([128, 4 * C], F32, tag="o")
                        o_bufs[jb0] = o_new
                    o_sb = o_bufs[jb0]
                    r_sb = xpool.tile([128, 1], F32, tag="r")
                    nc.vector.reciprocal(r_sb[:], zt[:, 64:65])
                    # normalize on ACT (per-partition scale), residual add on
                    # gpsimd (SBUF-only engine) - keeps the epilogue off the
                    # busier DVE.
                    nc.scalar.activation(
                        o_sb[:, s * C : (s + 1) * C], zt[:, 0:64],
                        mybir.ActivationFunctionType.Copy, scale=r_sb[:],
                    )
                    nc.gpsimd.tensor_add(
                        o_sb[:, s * C : (s + 1) * C],
                        o_sb[:, s * C : (s + 1) * C],
                        brest_sb[:, jb * C : (jb + 1) * C],
                    )
                    if s == nsub - 1:  # one store per jblock
                        nc.sync.dma_start(
                            out_d[:, (jb0 // 128) * C : (jb + 1) * C],
                            o_bufs.pop(jb0)[:, : nsub * C],
                        )

    nc.compile()
    return nc


def get_program(reps=1, dve_share=None):
    key = (reps, dve_share)
    if key not in _PROGRAMS:
        _PROGRAMS[key] = _build_program(reps, dve_share)
    return _PROGRAMS[key]


def make_in_maps(rv_x, bev_x, Wq, bq, Wk, bk, Wv, bv):
    rv_x = np.asarray(rv_x, np.float32)
    bev_x = np.asarray(bev_x, np.float32)
    ones = np.ones((1, N), np.float32)
    wq_ext = np.concatenate([np.asarray(Wq).T, np.asarray(bq)[None]], 0)
    wk_ext = np.concatenate([np.asarray(Wk).T, np.asarray(bk)[None]], 0)
    wv_ext = np.zeros((65, 65), np.float32)
    wv_ext[:64, :64] = np.asarray(Wv).T
    wv_ext[64, :64] = np.asarray(bv)
    wv_ext[64, 64] = 1.0
    w_ext = np.concatenate([wq_ext, wk_ext, wv_ext], 1).astype(BF16)

    in_maps = []
    for core in range(NCORES):
        b = core // 4
        j0 = (core % 4) * JS
        rv2 = rv_x[b].reshape(C, N)
        bev2 = bev_x[b].reshape(C, N)
        rv_ext = np.concatenate([rv2, ones], 0).astype(BF16)
        bev_ext = np.concatenate([bev2, ones], 0).astype(BF16)
        # residual pre-swizzled to the kernel's SBUF layout [p, (jblk, c)]
        brest = (
            bev2[:, j0 : j0 + JS].T.reshape(JS // 128, 128, C)
            .transpose(1, 0, 2).reshape(128, -1)
        )
        in_maps.append(
            {
                "rv_ext": rv_ext,
                "bev_ext": bev_ext,
                "rv_slab": np.ascontiguousarray(rv_ext[:, j0 : j0 + JS]),
                "bev_res_t": np.ascontiguousarray(brest),
                "w_ext": w_ext,
            }
        )
    return in_maps


def unswizzle_out(arr):
    """[128, (jblk, c)] device layout -> [C, JS] slab."""
    return arr.reshape(128, JS // 128, C).transpose(1, 0, 2).reshape(JS, C).T


def run(inputs, trace=False, trace_kwargs=None, reps=1, in_maps=None):
    """Run on all 8 cores; returns (output ndarray, BassKernelResults)."""
    from concourse.bass_utils import run_bass_kernel_spmd

    nc = get_program(reps)
    if in_maps is None:
        in_maps = make_in_maps(**inputs)
    res = run_bass_kernel_spmd(
        nc,
        in_maps,
        core_ids=list(range(NCORES)),
        trace=trace,
        **(trace_kwargs or {}),
    )
    out = np.zeros((B, C, N), np.float32)
    for core in range(NCORES):
        b = core // 4
        j0 = (core % 4) * JS
        out[b, :, j0 : j0 + JS] = unswizzle_out(res.results[core]["out"])
    return out.reshape(B, C, H, W), res


def kernel(**inputs):
    out, _ = run(inputs)
    return out
